# revision 1
# baseline (speedup 1.0000x reference)
"""CBoW embedding-bag kernel for Trainium2 (8 NeuronCores, batch-sharded).

Reference computation (see problem):
  - tokens [200, 1024] int32 in [0, 100000)
  - per batch column: sum embeddings of the *unique* tokens (first-occurrence
    dedup) from two tables lut/static_lut [100000, 300] f32
  - hidden = concat(e_learn, e_static) [B, 600]; h = relu(hidden @ W1.T + b1)
  - out = h @ W2.T + b2 -> [B]

Strategy:
  - Data parallel: 8 cores x 128 batch columns each; tables replicated.
  - Host prep: one fp16 table [100001, 600] = [lut | static_lut] rows plus a
    zero row at index 100000. Duplicate (non-first-occurrence) tokens are
    redirected on-device to the zero row, so a plain (unweighted) sum over all
    200 positions equals the dedup'd sum.
  - On device per core: dup detection via pairwise equality counts (only j<i
    lower half) on DVE, then 200 indirect-DMA gathers (one per sequence
    position, 128 rows each — the vector multi-offset DMA mode is broken on
    this runtime so one offset per partition is the only working shape),
    incremental pairwise-tree summation in fp16, and the W1 contraction as a
    PSUM-accumulated GEMM folded into each chunk so no serial tail remains.
  - The GpSimd (SWDGE) gather spine (~1.4us/call incl. dispatch) is the
    critical path; every other engine's work is emitted to overlap under it.
"""

import numpy as np

import concourse.bacc as bacc
import concourse.bass as bass
import concourse.mybir as mybir
import concourse.tile as tile
from concourse.bass_utils import run_bass_kernel_spmd

F16 = mybir.dt.float16
F32 = mybir.dt.float32
I32 = mybir.dt.int32

S = 200          # sequence length
B = 128          # batch columns per core
NCORES = 8
V = 100000       # vocab
D = 300          # per-table embedding dim
D2 = 600         # concat dim
# chunk sizes: small first chunk so the gather spine starts early, small last
# chunk so the post-spine reduction tail is short
CHUNKS = [8] + [24] * 7 + [16, 8]
assert sum(CHUNKS) == S
CK = max(CHUNKS)


def build_program():
    nc = bacc.Bacc("TRN2", target_bir_lowering=False, debug=False)

    tok_t = nc.dram_tensor("tok_t", [B, S], I32, kind="ExternalInput")
    tab = nc.dram_tensor("tab", [V + 1, D2], F16, kind="ExternalInput")
    w1t = nc.dram_tensor("w1t", [D2, D2], F16, kind="ExternalInput")
    b1 = nc.dram_tensor("b1", [1, D2], F32, kind="ExternalInput")
    w2 = nc.dram_tensor("w2", [1, D2], F32, kind="ExternalInput")
    b2 = nc.dram_tensor("b2", [1, 1], F32, kind="ExternalInput")
    out = nc.dram_tensor("out", [B, 1], F32, kind="ExternalOutput")

    AX = mybir.AxisListType
    OP = mybir.AluOpType

    with tile.TileContext(nc) as tc:
        with tc.tile_pool(name="const", bufs=1) as constp, \
             tc.tile_pool(name="io", bufs=1) as iop, \
             tc.tile_pool(name="mlp", bufs=1) as mlpp, \
             tc.tile_pool(name="maskp", bufs=2) as maskp, \
             tc.tile_pool(name="gatherp", bufs=4) as gatherp, \
             tc.tile_pool(name="treep", bufs=2) as treep, \
             tc.tile_pool(name="psum", bufs=2, space="PSUM") as psump:

            # ---------------- constants & inputs ----------------
            tok_i = iop.tile([B, S], I32)
            nc.sync.dma_start(out=tok_i[:], in_=tok_t.ap())
            tokf = iop.tile([B, S], F32)
            nc.vector.tensor_copy(out=tokf[:], in_=tok_i[:])

            # strict lower-triangle mask (i > j) and PE-transpose identity,
            # embedded as NEFF consts and loaded via HWDGE so the POOL queue
            # holds nothing but the gather spine
            tri_const = nc.inline_tensor(
                np.tril(np.ones((CK, CK), np.float16), -1).reshape(1, CK * CK),
                "tri_const")
            trimask = constp.tile([B, CK, CK], F16)
            nc.sync.dma_start(
                out=trimask[:].rearrange("p a b -> p (a b)"),
                in_=tri_const.ap().to_broadcast([B, CK * CK]))

            idn_const = nc.inline_tensor(np.eye(B, dtype=np.float16), "idn_const")
            idn16 = constp.tile([B, B], F16)
            nc.sync.dma_start(out=idn16[:], in_=idn_const.ap())

            vzero = constp.tile([B, 1], F32)
            nc.vector.memset(vzero[:], float(V))  # index of the zero row

            tokp = iop.tile([B, S], I32)   # redirected tokens

            # ---------------- dup-mask + redirect, one chunk at a time ------
            JW = 88  # block-compare j-slice width

            def mask_gen(c):
                """Generator emitting the dup-mask ops for chunk c one
                instruction at a time, so the caller can interleave them with
                other DVE work (spreads the DVE load that contends with SWDGE
                descriptor generation for SBUF ports)."""
                ck = CHUNKS[c]
                i0 = sum(CHUNKS[:c])
                isl = slice(i0, i0 + ck)

                # intra-chunk triangle counts
                eqtri = maskp.tile([B, CK, CK], F16, name="eqtri")
                nc.vector.tensor_tensor(
                    out=eqtri[:, :ck, :ck],
                    in0=tokf[:, isl].unsqueeze(2).to_broadcast([B, ck, ck]),
                    in1=tokf[:, isl].unsqueeze(1).to_broadcast([B, ck, ck]),
                    op=OP.is_equal,
                )
                yield
                nc.vector.tensor_tensor(
                    out=eqtri[:, :ck, :ck], in0=eqtri[:, :ck, :ck],
                    in1=trimask[:, :ck, :ck], op=OP.mult)
                yield
                cntc = maskp.tile([B, CK], F32, name="cntc")
                nc.vector.tensor_reduce(
                    out=cntc[:, :ck], in_=eqtri[:, :ck, :ck], axis=AX.X,
                    op=OP.add)
                yield

                # counts vs all earlier chunks (block part, j < i0), in
                # j-slices of <= JW to bound the eq scratch tile
                for j0 in range(0, i0, JW):
                    jw = min(JW, i0 - j0)
                    eqblk = maskp.tile([B, CK, JW], F16, name="eqblk")
                    nc.vector.tensor_tensor(
                        out=eqblk[:, :ck, :jw],
                        in0=tokf[:, isl].unsqueeze(2).to_broadcast([B, ck, jw]),
                        in1=tokf[:, j0:j0 + jw].unsqueeze(1).to_broadcast(
                            [B, ck, jw]),
                        op=OP.is_equal,
                    )
                    yield
                    blkcnt = maskp.tile([B, CK], F32, name="blkcnt")
                    nc.vector.tensor_reduce(
                        out=blkcnt[:, :ck], in_=eqblk[:, :ck, :jw], axis=AX.X,
                        op=OP.add)
                    yield
                    nc.vector.tensor_tensor(
                        out=cntc[:, :ck], in0=cntc[:, :ck], in1=blkcnt[:, :ck],
                        op=OP.add)
                    yield

                # dup -> redirect token to the zero row (fp32, then cast)
                isdup = maskp.tile([B, CK], I32, name="isdup")
                nc.vector.tensor_scalar(
                    out=isdup[:, :ck], in0=cntc[:, :ck], scalar1=0.0,
                    scalar2=None, op0=OP.is_gt)
                yield
                tokpf = maskp.tile([B, CK], F32, name="tokpf")
                nc.vector.tensor_copy(out=tokpf[:, :ck], in_=tokf[:, isl])
                yield
                nc.vector.copy_predicated(
                    out=tokpf[:, :ck], mask=isdup[:, :ck],
                    data=vzero[:].to_broadcast([B, ck]))
                yield
                nc.vector.tensor_copy(out=tokp[:, isl], in_=tokpf[:, :ck])

            def emit_mask(c):
                for _ in mask_gen(c):
                    pass

            # masks for the first chunks up-front; the rest are emitted a few
            # chunks ahead inside the spine loop so the DVE load (which
            # contends with SWDGE descriptor generation for SBUF ports)
            # spreads out instead of bursting at the start
            MASK_AHEAD = 3
            for c in range(min(MASK_AHEAD, len(CHUNKS))):
                emit_mask(c)

            # MLP weights / vectors (needed only at the GEMM stage)
            w2rep = constp.tile([B, D2], F32)
            nc.sync.dma_start(out=w2rep[:], in_=w2.ap().to_broadcast([B, D2]))
            b1rep = constp.tile([B, D2], F32)
            nc.sync.dma_start(out=b1rep[:], in_=b1.ap().to_broadcast([B, D2]))
            b2rep = constp.tile([B, 1], F32)
            nc.sync.dma_start(out=b2rep[:], in_=b2.ap().to_broadcast([B, 1]))

            w1sb = []
            for ki in range(6):
                w1k = mlpp.tile([100, D2], F16, name=f"w1k{ki}")
                nc.sync.dma_start(out=w1k[:], in_=w1t.ap()[100 * ki:100 * (ki + 1), :])
                w1sb.append(w1k)

            # ---------------- phase 2: gather spine + tree sums + chunk GEMM
            # h1_pre accumulates in PSUM across chunks so the W1 contraction
            # overlaps the gather spine instead of being a serial tail.
            ph = [psump.tile([B, D], F32, name=f"ph{nh}", bufs=1)
                  for nh in range(2)]
            flat = lambda ap: ap.rearrange("p a b -> p (a b)")
            for c, ck in enumerate(CHUNKS):
                i0 = sum(CHUNKS[:c])
                # one indirect call per s position (128 rows each)
                G = gatherp.tile([B, CK, D2], F16, name="G")
                for k in range(ck):
                    # s=0 is never a duplicate: gather it straight from the
                    # raw tokens so the spine starts before any mask work
                    off_src = tok_i if (c == 0 and k == 0) else tokp
                    nc.gpsimd.indirect_dma_start(
                        out=G[:, k, :], out_offset=None,
                        in_=tab.ap(),
                        in_offset=bass.IndirectOffsetOnAxis(
                            ap=off_src[:, i0 + k:i0 + k + 1], axis=0),
                    )

                # mask ops for chunk c+MASK_AHEAD, interleaved between the
                # pair ops of this chunk: each pair waits for its gathers to
                # land, so the mask ops spread across the chunk window
                # instead of bursting at its start
                mg = (mask_gen(c + MASK_AHEAD)
                      if c + MASK_AHEAD < len(CHUNKS) else None)

                # incremental pairwise tree: pair ops become ready as the
                # gathers land, so little reduction is left after the
                # chunk's last gather.
                m = ck // 2
                pr = treep.tile([B, CK // 2, D2], F16, name="pr")
                for j in range(m):
                    nc.vector.tensor_tensor(
                        out=pr[:, j, :], in0=G[:, 2 * j, :],
                        in1=G[:, 2 * j + 1, :], op=OP.add)
                    if mg is not None:
                        next(mg, None)
                        next(mg, None)
                if mg is not None:
                    for _ in mg:
                        pass
                n = m
                while n % 2 == 0 and n > 1:
                    h = n // 2
                    nc.vector.tensor_tensor(
                        out=flat(pr[:, 0:h, :]), in0=flat(pr[:, 0:h, :]),
                        in1=flat(pr[:, h:n, :]), op=OP.add)
                    n = h
                if n == 3:
                    csum = treep.tile([B, D2], F16, name="csum")
                    nc.vector.tensor_tensor(
                        out=csum[:], in0=pr[:, 0, :], in1=pr[:, 1, :],
                        op=OP.add)
                    nc.vector.tensor_tensor(
                        out=csum[:], in0=csum[:], in1=pr[:, 2, :], op=OP.add)
                    csum_ap = csum[:]
                else:
                    assert n == 1
                    csum_ap = pr[:, 0, :]

                # transpose chunk sum (6 x [128,100] -> [100,128]) and fold
                # into the PSUM-accumulated h1_pre GEMM (all fp16). Batch the
                # six transposes, then the copies, then the matmuls — the
                # per-ki transpose->copy->matmul ladder costs ~1us of
                # cross-engine sem latency per hop, which the last chunks
                # cannot hide. Copies go to DVE at the tail (idle there).
                pts = []
                for ki in range(6):
                    pt = psump.tile([B, B], F16, name="pt", bufs=6)
                    nc.tensor.transpose(
                        out=pt[0:100, :],
                        in_=csum_ap[:, 100 * ki:100 * (ki + 1)],
                        identity=idn16[:],
                    )
                    pts.append(pt)
                hks = []
                for ki in range(6):
                    hk = mlpp.tile([100, B], F16, name=f"h0T{ki}", bufs=2)
                    if c >= len(CHUNKS) - 2:
                        nc.vector.tensor_copy(out=hk[:], in_=pts[ki][0:100, :])
                    else:
                        nc.scalar.copy(out=hk[:], in_=pts[ki][0:100, :])
                    hks.append(hk)
                for ki in range(6):
                    for nh in range(2):
                        nc.tensor.matmul(
                            out=ph[nh][:],
                            lhsT=hks[ki][:],
                            rhs=w1sb[ki][:, D * nh:D * (nh + 1)],
                            start=(c == 0 and ki == 0),
                            stop=(c == len(CHUNKS) - 1 and ki == 5),
                        )

            # ---------------- MLP tail ----------------
            h1 = mlpp.tile([B, D2], F32)
            for nh in range(2):
                nsl = slice(D * nh, D * (nh + 1))
                nc.vector.tensor_tensor(
                    out=h1[:, nsl], in0=ph[nh][:], in1=b1rep[:, nsl], op=OP.add)
            nc.vector.tensor_scalar(
                out=h1[:], in0=h1[:], scalar1=0.0, scalar2=None, op0=OP.max)

            # out = h1 . W2 + b2
            prod = mlpp.tile([B, D2], F32)
            dot = mlpp.tile([B, 1], F32)
            nc.vector.scalar_tensor_tensor(
                out=prod[:], in0=h1[:], scalar=1.0, op0=OP.mult,
                in1=w2rep[:], op1=OP.mult, accum_out=dot[:])
            outsb = mlpp.tile([B, 1], F32)
            nc.vector.tensor_tensor(
                out=outsb[:], in0=dot[:], in1=b2rep[:], op=OP.add)
            nc.sync.dma_start(out=out.ap(), in_=outsb[:])

    nc.compile()
    return nc


_NC = None


def _get_program():
    global _NC
    if _NC is None:
        _NC = build_program()
    return _NC


def make_inputs(tokens, lut, static_lut, W1, b1, W2, b2):
    """Host-side prep: shard tokens, build the padded fp16 concat table."""
    tokens = np.asarray(tokens)
    tokens_t = np.ascontiguousarray(tokens.T).astype(np.int32, copy=False)
    tab = np.zeros((V + 1, D2), np.float16)
    tab[:V, :D] = np.asarray(lut, dtype=np.float16)
    tab[:V, D:] = np.asarray(static_lut, dtype=np.float16)
    w1t = np.ascontiguousarray(np.asarray(W1, dtype=np.float16).T)
    b1v = np.asarray(b1, dtype=np.float32).reshape(1, D2)
    w2v = np.asarray(W2, dtype=np.float32).reshape(1, D2)
    b2v = np.asarray(b2, dtype=np.float32).reshape(1, 1)
    in_maps = []
    for i in range(NCORES):
        in_maps.append({
            "tok_t": tokens_t[i * B:(i + 1) * B],
            "tab": tab,
            "w1t": w1t,
            "b1": b1v,
            "w2": w2v,
            "b2": b2v,
        })
    return in_maps


def kernel(tokens, lut, static_lut, W1, b1, W2, b2, _trace=False, _trace_kwargs=None):
    nc = _get_program()
    in_maps = make_inputs(tokens, lut, static_lut, W1, b1, W2, b2)
    res = run_bass_kernel_spmd(
        nc, in_maps, core_ids=list(range(NCORES)),
        trace=_trace, **(_trace_kwargs or {}))
    out = np.concatenate([res.results[i]["out"][:, 0] for i in range(NCORES)])
    if _trace:
        kernel._last_results = res
    return out

